# revision 31
# baseline (speedup 1.0000x reference)
"""Trainium2 Bass kernel for AttentionIn: fused QKV projection + bias + GPT-J rotary.

Reference computation (all fp32):
    q = einsum('bpd,hde->bphe', x, W_Q) + b_Q   (same for k, v)
    q, k = rotary(q), rotary(k)   # GPT-J interleaved, first 64 dims of each head
    returns (residual, q.reshape(b,s,2048), k..., v...)

Sharding: data-parallel over the flattened (batch*seq) axis — 8192 rows split
as 1024 rows per core. Each core computes all 16 heads of q/k/v for its rows:
one [1024, 2048] @ [2048, 6144] matmul (Q|K|V stacked along columns). The bias
add is fused into the PSUM->SBUF move on the vector engine (bias replicated
across partitions host-side), and GPT-J rotary is applied in SBUF via
strided/pair-swapped access patterns with host-precomputed sin/cos tables.

x is transposed on the host (xT layout [d_model, rows]) so the contraction dim
lands on SBUF partitions with no on-chip transposes. Matmuls run in float32r
(full PE rate at N>=256, ~tf32-or-better accuracy: measured ~1.5e-4
scale-relative absmax vs the fp32 reference).
"""
import sys

sys.path.insert(0, '/opt/trn_rl_repo')

import numpy as np
import concourse.mybir as mybir
import concourse.tile as tile
from concourse import bacc
from concourse.bass_utils import run_bass_kernel_spmd
from contextlib import ExitStack

P = 128
N_HEADS = 16
D_MODEL = 2048
D_HEAD = 128
ROT = 64
BATCH = 4
SEQ = 2048
ROTARY_BASE = 10000.0

NCORES = 8
ROWS = BATCH * SEQ            # 8192
RPC = ROWS // NCORES          # 1024 rows per core
NT = RPC // P                 # 8 pos-tiles per core
KC = D_MODEL // P             # 16 k-chunks
COLS = 3 * N_HEADS * D_HEAD   # 6144 output cols (Q|K|V)
BLK = 512                     # output col-block (one PSUM bank, 4 heads)
NBLK = COLS // BLK            # 12
HPB = BLK // D_HEAD           # heads per block = 4

F32R = mybir.dt.float32r
F32 = mybir.dt.float32

_CACHE = {}


def _build(loop_iters=None):
    """Build the per-core Bass program. loop_iters wraps the whole body in a
    hardware loop (timing-only variant; the production kernel passes None)."""
    nc = bacc.Bacc()
    xt_d = nc.declare_dram_parameter("xt", [D_MODEL, RPC], F32R, isOutput=False)
    # W pre-arranged host-side as [blk, k, p, n] so each per-(blk,k) chunk DMA
    # is one fully contiguous 256 KB read
    w_d = nc.declare_dram_parameter("w", [NBLK, KC, P, BLK], F32R, isOutput=False)
    bias_d = nc.declare_dram_parameter("bias", [P, COLS], F32, isOutput=False)
    sin_d = nc.declare_dram_parameter("sin", [RPC, ROT], F32, isOutput=False)
    cos_d = nc.declare_dram_parameter("cos", [RPC, ROT], F32, isOutput=False)
    qo_d = nc.declare_dram_parameter("qo", [RPC, N_HEADS * D_HEAD], F32, isOutput=True)
    ko_d = nc.declare_dram_parameter("ko", [RPC, N_HEADS * D_HEAD], F32, isOutput=True)
    vo_d = nc.declare_dram_parameter("vo", [RPC, N_HEADS * D_HEAD], F32, isOutput=True)
    outs = [qo_d, ko_d, vo_d]

    xt_r = xt_d[:].rearrange("(kc p) t -> p kc t", p=P)    # [128, KC, RPC]
    sin_r = sin_d[:].rearrange("(t p) j -> p t j", p=P)    # [128, NT, ROT]
    cos_r = cos_d[:].rearrange("(t p) j -> p t j", p=P)

    with tile.TileContext(nc) as tc, ExitStack() as ctx:
        const = ctx.enter_context(tc.tile_pool(name="const", bufs=1))
        wpool = ctx.enter_context(tc.tile_pool(name="wpool", bufs=18))
        obuf = ctx.enter_context(tc.tile_pool(name="obuf", bufs=12))
        tmpp = ctx.enter_context(tc.tile_pool(name="tmpp", bufs=8))
        psum = ctx.enter_context(tc.tile_pool(name="psum", bufs=8, space="PSUM"))

        def body():
            # xt chunks on the gpsimd queue, one TILE per chunk: Tile tracks
            # deps per tile, so k=0 matmuls start as soon as chunk 0 lands
            # (~2us) — and the sync queue stays free for the W stream, whose
            # first chunk gates the very first matmul
            # chunk 0 further split in four so the very first matmuls wait on
            # 128 KB, not 512 KB
            xt0_parts = []
            for j in range(4):
                xp = const.tile([P, RPC // 4], F32R, tag=f"xt0_{j}", name=f"xt0_{j}")
                nc.gpsimd.dma_start(xp[:], xt_r[:, 0, j * (RPC // 4):(j + 1) * (RPC // 4)])
                xt0_parts.append(xp)
            xt_sb = [None]
            for k in range(1, KC):
                xt_k = const.tile([P, RPC], F32R, tag=f"xt{k}", name=f"xt{k}")
                nc.gpsimd.dma_start(xt_k[:], xt_r[:, k])
                xt_sb.append(xt_k)

            def xt_ap(k, t):
                if k == 0:
                    return xt0_parts[t // 2][:, (t % 2) * P:(t % 2 + 1) * P]
                return xt_sb[k][:, t * P:(t + 1) * P]
            # constants on the (otherwise idle) scalar-engine DMA queue — off
            # both critical queues: sync carries the W stream, gpsimd the xt
            # chunks
            bias_sb = const.tile([P, COLS], F32, tag="bias")
            nc.scalar.dma_start(bias_sb[:], bias_d[:])
            sin_sb = const.tile([P, NT, ROT], F32, tag="sin")
            cos_sb = const.tile([P, NT, ROT], F32, tag="cos")
            nc.scalar.dma_start(sin_sb[:], sin_r)
            nc.scalar.dma_start(cos_sb[:], cos_r)

            def emit_gang(blk, proj, c0, col, tiles, w_tiles=None):
                # k-outer accumulation, one PSUM bank per pos-tile in the gang
                pss = [psum.tile([P, BLK], F32, name=f"ps{t}", tag="ps")
                       for t in tiles]
                fetched = []
                for k in range(KC):
                    if w_tiles is None:
                        w_sb = wpool.tile([P, BLK], F32R, tag="w")
                        nc.sync.dma_start(w_sb[:], w_d[blk, k])
                        fetched.append(w_sb)
                    else:
                        w_sb = w_tiles[k]
                    for i, t in enumerate(tiles):
                        nc.tensor.matmul(pss[i][:], xt_ap(k, t),
                                         w_sb[:], start=(k == 0), stop=(k == KC - 1))
                # all bank-releasing bias-adds FIRST (the next gang's matmuls
                # wait on these), rotary + stores after
                obs = []
                for i, t in enumerate(tiles):
                    ob = obuf.tile([P, BLK], F32, tag="ob", name=f"ob{t}")
                    # bias add fused into the PSUM->SBUF move (bias replicated
                    # across partitions host-side)
                    nc.vector.tensor_add(ob[:], pss[i][:], bias_sb[:, c0:c0 + BLK])
                    obs.append(ob)
                for i, t in enumerate(tiles):
                    ob = obs[i]
                    if proj < 2:  # rotary for q and k (reads ob, not psum, so
                        # the bank freed after the single add above)
                        for h in range(HPB):
                            base = h * D_HEAD
                            rot = ob[:, base:base + ROT]
                            ob_swap = ob[:, base:base + ROT].rearrange(
                                "p (a two) -> p a two", two=2)[:, :, ::-1]
                            tmp = tmpp.tile([P, ROT], F32, tag="tmp")
                            nc.vector.tensor_mul(
                                tmp[:].rearrange("p (a two) -> p a two", two=2),
                                ob_swap,
                                sin_sb[:, t].rearrange("p (a two) -> p a two", two=2))
                            nc.vector.tensor_mul(rot, rot, cos_sb[:, t])
                            nc.vector.tensor_add(rot, rot, tmp[:])
                    nc.sync.dma_start(outs[proj][t * P:(t + 1) * P, col:col + BLK],
                                      ob[:])
                return fetched

            for blk in range(NBLK):
                proj = blk // (NBLK // 3)          # 0=q, 1=k, 2=v
                c0 = blk * BLK
                col = (blk % (NBLK // 3)) * BLK
                if blk == NBLK - 1:
                    # last block in shrinking gangs (4,2,2): each gang's DVE
                    # epilogue hides under the next gang's matmuls, so the
                    # kernel-tail serial epilogue is only 2 tiles deep
                    wt = emit_gang(blk, proj, c0, col, range(4))
                    emit_gang(blk, proj, c0, col, range(4, 6), w_tiles=wt)
                    emit_gang(blk, proj, c0, col, range(6, NT), w_tiles=wt)
                else:
                    emit_gang(blk, proj, c0, col, range(NT))

        if loop_iters is None:
            body()
        else:
            with tc.For_i(0, loop_iters, 1):
                body()
    nc.finalize()
    return nc


def _prep_inputs(residual, x, W_Q, W_K, W_V, b_Q, b_K, b_V):
    """Host-side prep: per-core in_maps for run_bass_kernel_spmd."""
    x = np.asarray(x, np.float32).reshape(ROWS, D_MODEL)
    w = np.concatenate(
        [np.asarray(W, np.float32).transpose(1, 0, 2).reshape(D_MODEL, N_HEADS * D_HEAD)
         for W in (W_Q, W_K, W_V)], axis=1)          # [2048, 6144]
    # [blk, k, p, n] layout: each per-(blk,k) [128,512] chunk is contiguous
    w = np.ascontiguousarray(
        w.reshape(KC, P, NBLK, BLK).transpose(2, 0, 1, 3))
    bcat = np.concatenate([np.asarray(b, np.float32).ravel()
                           for b in (b_Q, b_K, b_V)])
    bias_full = np.ascontiguousarray(np.broadcast_to(bcat, (P, COLS)))

    # GPT-J rotary tables (interleaved; sin pre-signed for the swap trick)
    pos = np.arange(SEQ, dtype=np.float32)
    dim = np.arange(ROT // 2, dtype=np.float32)
    freq = ROTARY_BASE ** (dim / (ROT / 2))
    angles = pos[:, None] / freq[None, :]            # [SEQ, 32]
    sin_i = np.repeat(np.sin(angles), 2, axis=1).astype(np.float32)   # [SEQ, 64]
    cos_i = np.repeat(np.cos(angles), 2, axis=1).astype(np.float32)
    sin_signed = sin_i * np.tile(np.array([-1.0, 1.0], np.float32), ROT // 2)

    in_maps = []
    for c in range(NCORES):
        xc = x[c * RPC:(c + 1) * RPC]
        p0 = (c * RPC) % SEQ
        in_maps.append({
            "xt": np.ascontiguousarray(xc.T),
            "w": w,
            "bias": bias_full,
            "sin": np.ascontiguousarray(sin_signed[p0:p0 + RPC]),
            "cos": np.ascontiguousarray(cos_i[p0:p0 + RPC]),
        })
    return in_maps


def _assemble(results):
    q = np.empty((ROWS, N_HEADS * D_HEAD), np.float32)
    k = np.empty((ROWS, N_HEADS * D_HEAD), np.float32)
    v = np.empty((ROWS, N_HEADS * D_HEAD), np.float32)
    for c in range(NCORES):
        q[c * RPC:(c + 1) * RPC] = results[c]["qo"]
        k[c * RPC:(c + 1) * RPC] = results[c]["ko"]
        v[c * RPC:(c + 1) * RPC] = results[c]["vo"]
    shp = (BATCH, SEQ, N_HEADS * D_HEAD)
    return q.reshape(shp), k.reshape(shp), v.reshape(shp)


def kernel(residual, x, W_Q, W_K, W_V, b_Q, b_K, b_V):
    if "nc" not in _CACHE:
        _CACHE["nc"] = _build()
    nc = _CACHE["nc"]
    in_maps = _prep_inputs(residual, x, W_Q, W_K, W_V, b_Q, b_K, b_V)
    res = run_bass_kernel_spmd(nc, in_maps, list(range(NCORES)))
    q, k, v = _assemble(res.results)
    return (np.asarray(residual, np.float32), q, k, v)
